# revision 1
# baseline (speedup 1.0000x reference)
"""nn_CSRSparseRetrievalModel: CSR sparse retrieval (SPLADE-style top-k)
as a Bass/Tile kernel for Trainium2, sharded across 8 NeuronCores.

Sharding (per spec hint): documents are sharded 8 ways; each core streams its
(indice, values) shard, computes the local SpMV against the replicated query
(32 compare-multiply term passes on the Vector engine, query terms baked in as
instruction immediates), reduces per-document scores on-device via dma_gather
windows + a custom zero-fill windowed-sum DVE op, and returns local scores.
The host merges the 8 cores' scores, rescores the top candidates exactly in
fp32, and emits the global top-k (values, indices).

kernel(**inputs) -> (top_values float32 [k], top_indices int32 [k])
"""


import numpy as np

import concourse.bass as bass
import concourse.tile as tile
from concourse import bacc, mybir
from concourse.alu_op_type import AluOpType

# ---------------------------------------------------------------------------
# Custom DVE op: zero-filled windowed sum
# ---------------------------------------------------------------------------

_WINDOW_SUM = None


def get_window_sum_op():
    """Register (once) and return the WINDOW_SUM_ANT custom DVE op.

    accum_out[p] = s1 + sum_k select(s0[p] <= k < in1[p], in0[p, k], 0)
    """
    global _WINDOW_SUM
    if _WINDOW_SUM is not None:
        return _WINDOW_SUM

    import concourse.dve_ops as dve_ops
    from concourse.dve_ops import DveOp, _accum_ref
    from concourse.dve_spec import (
        Spec, Src0, C0, C1, C3, Zero, select, lower, _spill_c3_to_src1,
    )
    from concourse.dve_spec import Idx, AluOp as SpecAluOp
    from concourse.dve_uop import DveOpSpec
    from concourse.dve_table_gen import dve_ver_for

    name = "WINDOW_SUM_ANT"
    if name in dve_ops._SUB_OPCODE_FOR_NAME:
        _WINDOW_SUM = next(op for op in dve_ops.OPS if op.name == name)
        return _WINDOW_SUM

    add = dve_ops.add  # AluOp.ADD helper already imported in dve_ops

    def _ref(in0, in1, c0, c1, c2):
        P = in0.shape[0]
        x = in0.astype(np.float32).reshape(P, -1)
        N = x.shape[1]
        idx = np.broadcast_to(np.arange(N, dtype=np.float32), (P, N))
        start = np.broadcast_to(np.asarray(c0, np.float32).reshape(-1, 1), (P, 1))
        end = np.asarray(in1, np.float32).reshape(P, 1)
        mask = (idx >= start) & (idx < end)
        body = np.where(mask, x, 0.0).astype(np.float32)
        return body, _accum_ref(body, c1, add, False)

    body = _spill_c3_to_src1(select((Idx >= C0) & (Idx < C3), Src0, Zero))
    spec = Spec(body=body, accum=add, accum_init=C1, reference=_ref)

    row = dve_ops._CUSTOM_DVE_ROW_BASE + len(dve_ops.OPS)
    assert row < 0x20, "no free custom-DVE rows"

    # compute the sha for this spec (normally pinned by hand)
    shas = {}
    for ver in ("v3", "v4"):
        try:
            uops = lower(spec, ver=ver)
        except Exception:
            continue
        from concourse.dve_spec import _has_src1
        tmp = DveOpSpec(name=name, opcode=row, uops=uops, rd1_en=_has_src1(spec))
        shas[ver] = tmp.sha(ver)

    op = DveOp(name, spec, subdim=False, uops_sha=shas)
    dve_ops.OPS.append(op)
    dve_ops._SUB_OPCODE_FOR_NAME[name] = row
    dve_ops.CUSTOM_DVE_SPECS[name] = spec
    _WINDOW_SUM = op
    return op


# ---------------------------------------------------------------------------
# Host-side planning
# ---------------------------------------------------------------------------

class Plan:
    pass


def make_plan(crow, q_indices, q_values, vocab, n_cores=8, n_sec=5, nf_align=64):
    """Compute shard/section geometry + per-core input tensors.

    crow: int64 [n_docs+1]; q_indices/q_values: [1, QNNZ]
    """
    p = Plan()
    n_docs = crow.shape[0] - 1
    assert n_docs % n_cores == 0
    p.n_cores = n_cores
    p.n_docs = n_docs
    p.dpc = n_docs // n_cores
    p.n_sec = n_sec
    rows_per_sec = -(-p.dpc // n_sec)  # ceil
    p.rows_per_sec = rows_per_sec
    p.rsec = -(-rows_per_sec // 128) * 128  # padded row slots per section

    # coalesce query terms
    qi = np.asarray(q_indices).reshape(-1).astype(np.int64)
    qv = np.asarray(q_values).reshape(-1).astype(np.float64)
    dense = np.zeros(vocab, dtype=np.float64)
    np.add.at(dense, qi, qv)
    nz = np.nonzero(dense)[0]
    assert len(nz) <= 32
    p.terms = [(int(i), float(np.float32(dense[i]))) for i in nz]
    p.query_dense = dense.astype(np.float32)

    # section spans
    crow = np.asarray(crow).astype(np.int64)
    # row ranges per (core, sec)
    sec_row_lo = np.zeros((n_cores, n_sec), dtype=np.int64)
    sec_row_hi = np.zeros((n_cores, n_sec), dtype=np.int64)
    max_span = 0
    for c in range(n_cores):
        base_row = c * p.dpc
        for s in range(n_sec):
            r0 = base_row + min(s * rows_per_sec, p.dpc)
            r1 = base_row + min((s + 1) * rows_per_sec, p.dpc)
            sec_row_lo[c, s] = r0
            sec_row_hi[c, s] = r1
            span = int(crow[r1] - crow[r0])
            max_span = max(max_span, span)
    # supertile free-dim: 2 supertiles of [128, NF] per section
    nf = -(-max_span // (2 * 128 * nf_align)) * nf_align
    nf = max(nf, nf_align)
    p.nf = nf
    p.scap = 2 * 128 * nf
    p.nsup = 2
    assert p.scap >= max_span
    n_blk_tab = (p.scap + 256) // 64  # gather table rows incl. overhang pad
    assert p.scap // 64 <= 32767, "section span exceeds int16 gather range"
    p.n_blk_tab = n_blk_tab
    p.sec_row_lo = sec_row_lo
    p.sec_row_hi = sec_row_hi
    return p


def make_core_inputs(p, crow, indice, values):
    """Build per-core input arrays (ind32, val32, gidx, rs, re)."""
    crow = np.asarray(crow).astype(np.int64)
    ins = []
    for c in range(p.n_cores):
        import ml_dtypes
        ind16 = np.zeros((p.n_sec, p.scap), dtype=np.int16)
        valbf = np.zeros((p.n_sec, p.scap), dtype=ml_dtypes.bfloat16)
        gidx = np.zeros((p.n_sec, 128, p.rsec // 16), dtype=np.int16)
        rs = np.zeros((p.n_sec, 128, p.rsec // 128), dtype=np.float32)
        re = np.zeros((p.n_sec, 128, p.rsec // 128), dtype=np.float32)
        for s in range(p.n_sec):
            r0, r1 = int(p.sec_row_lo[c, s]), int(p.sec_row_hi[c, s])
            e0, e1 = int(crow[r0]), int(crow[r1])
            span = e1 - e0
            ind16[s, :span] = indice[e0:e1].astype(np.int16)
            valbf[s, :span] = values[e0:e1].astype(ml_dtypes.bfloat16)
            nrows = r1 - r0
            starts = (crow[r0:r1] - e0).astype(np.int64)
            ends = (crow[r0 + 1:r1 + 1] - e0).astype(np.int64)
            blk = starts >> 6
            rs_rel = (starts - (blk << 6)).astype(np.float32)
            re_rel = (ends - (blk << 6)).astype(np.float32)
            assert re_rel.size == 0 or re_rel.max() <= 256
            # wrapped idx layout [16, rsec//16] replicated to 128 partitions
            blk_pad = np.zeros(p.rsec, dtype=np.int16)
            blk_pad[:nrows] = blk.astype(np.int16)
            wrapped = blk_pad.reshape(p.rsec // 16, 16).T  # [16, rsec//16]
            gidx[s] = np.tile(wrapped, (8, 1))
            # bounds layout [128, rsec//128]: row i at [i%128, i//128]
            rs_pad = np.zeros(p.rsec, dtype=np.float32)
            re_pad = np.zeros(p.rsec, dtype=np.float32)  # empty rows: [0,0)
            rs_pad[:nrows] = rs_rel
            re_pad[:nrows] = re_rel
            rs[s] = rs_pad.reshape(p.rsec // 128, 128).T
            re[s] = re_pad.reshape(p.rsec // 128, 128).T
        ins.append(dict(ind16=ind16, valbf=valbf, gidx=gidx, rs=rs, re=re))
    return ins


# ---------------------------------------------------------------------------
# Device program
# ---------------------------------------------------------------------------

def build_program(p, n_devices=8):
    nc = bacc.Bacc("TRN2", target_bir_lowering=False, debug=False,
                   num_devices=n_devices)
    f32, bf16 = mybir.dt.float32, mybir.dt.bfloat16
    i32, i16 = mybir.dt.int32, mybir.dt.int16

    ind16 = nc.declare_dram_parameter("ind16", [p.n_sec, p.scap], i16, isOutput=False)
    valbf = nc.declare_dram_parameter("valbf", [p.n_sec, p.scap], bf16, isOutput=False)
    gidx = nc.declare_dram_parameter("gidx", [p.n_sec, 128, p.rsec // 16], i16, isOutput=False)
    rsb = nc.declare_dram_parameter("rs", [p.n_sec, 128, p.rsec // 128], f32, isOutput=False)
    reb = nc.declare_dram_parameter("re", [p.n_sec, 128, p.rsec // 128], f32, isOutput=False)
    scores_out = nc.declare_dram_parameter("scores", [p.n_sec, 128, p.rsec // 128], f32, isOutput=True)

    # prod scratch in DRAM, per section, +256 elem overhang for window reads
    prod_dram = nc.dram_tensor("prod_scratch", [p.n_sec, p.scap + 256], f32)

    wsum = get_window_sum_op()
    nblk = p.rsec // 128

    with tile.TileContext(nc) as tc:
        with tc.tile_pool(name="io", bufs=2) as io_pool, \
             tc.tile_pool(name="acc", bufs=1) as acc_pool, \
             tc.tile_pool(name="work", bufs=2) as work_pool, \
             tc.tile_pool(name="gath", bufs=2) as gath_pool, \
             tc.tile_pool(name="small", bufs=2) as small_pool:

            # zero the per-section window overhang once
            ztile = small_pool.tile([128, 2], f32, tag="zero")
            nc.vector.memset(ztile[:], 0.0)
            for s in range(p.n_sec):
                nc.sync.dma_start(
                    prod_dram[s, p.scap:p.scap + 256].rearrange(
                        "(q f) -> q f", f=2), ztile[:])

            # optional whole-pipeline repeat (timing amplification; idempotent)
            for _rep in range(getattr(p, 'repeat', 1)):
                for s in range(p.n_sec):
                    # ---- Phase A: match + prod ----
                    for k in range(p.nsup):
                        seg = slice(k * 128 * p.nf, (k + 1) * 128 * p.nf)
                        ind_t = io_pool.tile([128, p.nf], i16, tag="ind")
                        nc.sync.dma_start(
                            ind_t[:], ind16[s, seg].rearrange("(q f) -> q f", f=p.nf))
                        val_t = io_pool.tile([128, p.nf], bf16, tag="val")
                        nc.sync.dma_start(
                            val_t[:], valbf[s, seg].rearrange("(q f) -> q f", f=p.nf))

                        acc = acc_pool.tile([128, p.nf], bf16, tag="acc")
                        tmp = acc_pool.tile([128, p.nf], bf16, tag="tmp")
                        terms_iter = list(p.terms) * getattr(p, 'dup_terms', 1)
                        n_gps = min(getattr(p, 'gps_terms', 0), max(0, len(terms_iter) - 2))
                        gps_terms = terms_iter[:n_gps]
                        dve_terms = terms_iter[n_gps:]
                        # GPSIMD computes its own partial max-accumulation
                        gacc = None
                        gtmp = None
                        if n_gps:
                            gacc = acc_pool.tile([128, p.nf], bf16, tag="gacc")
                            gtmp = acc_pool.tile([128, p.nf], bf16, tag="gtmp")
                        for i, (tqi, tqv) in enumerate(gps_terms):
                            if i == 0:
                                nc.gpsimd.tensor_scalar(
                                    gacc[:], ind_t[:], float(tqi), float(tqv),
                                    AluOpType.is_equal, AluOpType.mult)
                            else:
                                nc.gpsimd.tensor_scalar(
                                    gtmp[:], ind_t[:], float(tqi), float(tqv),
                                    AluOpType.is_equal, AluOpType.mult)
                                nc.gpsimd.tensor_tensor(
                                    gacc[:], gacc[:], gtmp[:], AluOpType.max)
                        variant = getattr(p, 'variant', 'a')
                        for i, (tqi, tqv) in enumerate(dve_terms):
                            if i == 0:
                                nc.vector.tensor_scalar(
                                    acc[:], ind_t[:], float(tqi), float(tqv),
                                    AluOpType.is_equal, AluOpType.mult)
                            elif variant == 'b':
                                nc.vector.tensor_scalar(
                                    tmp[:], ind_t[:], float(tqi), None,
                                    AluOpType.is_equal)
                                nc.vector.scalar_tensor_tensor(
                                    acc[:], tmp[:], float(tqv), acc[:],
                                    AluOpType.mult, AluOpType.add)
                            else:
                                nc.vector.tensor_scalar(
                                    tmp[:], ind_t[:], float(tqi), float(tqv),
                                    AluOpType.is_equal, AluOpType.mult)
                                nc.vector.tensor_tensor(
                                    acc[:], acc[:], tmp[:], AluOpType.max)
                        if n_gps:
                            nc.vector.tensor_tensor(
                                acc[:], acc[:], gacc[:], AluOpType.max)
                        prod_t = work_pool.tile([128, p.nf], bf16, tag="prod")
                        nc.vector.tensor_tensor(prod_t[:], acc[:], val_t[:], AluOpType.mult)
                        # cast bf16 -> f32 during the DMA out (SWDGE)
                        nc.gpsimd.dma_start(
                            prod_dram[s, seg].rearrange("(q f) -> q f", f=p.nf), prod_t[:])

                gch = min(getattr(p, 'gch', 8), nblk)  # gather chunk: blocks per dma_gather
                for s in range(p.n_sec):
                    # ---- Phase B: gather windows + windowed sums ----
                    gix_t = small_pool.tile([128, p.rsec // 16], i16, tag="gix")
                    nc.sync.dma_start(gix_t[:], gidx[s])
                    rs_t = small_pool.tile([128, nblk], f32, tag="rs")
                    nc.sync.dma_start(rs_t[:], rsb[s])
                    re_t = small_pool.tile([128, nblk], f32, tag="re")
                    nc.sync.dma_start(re_t[:], reb[s])

                    table = prod_dram[s, :].copy()
                    table.ap = mybir.VecI64Pair([(64, p.scap // 64), (1, 256)])
                    sc_t = small_pool.tile([128, nblk], f32, tag="sc")
                    junk = gath_pool.tile([128, 256], f32, tag="junk")
                    for b0 in range(0, nblk, gch):
                        nb = min(gch, nblk - b0)
                        win_t = gath_pool.tile([128, gch, 256], f32, tag="win")
                        nc.gpsimd.dma_gather(
                            win_t[:, :nb, :], table,
                            gix_t[:, b0 * 8:(b0 + nb) * 8],
                            num_idxs=nb * 128, num_idxs_reg=nb * 128,
                            elem_size=256, elem_step=64)
                        for b in range(nb):
                            nc.vector._custom_dve(
                                wsum,
                                out=junk[:],
                                in0=win_t[:, b, :],
                                in1=re_t[:, b0 + b:b0 + b + 1],
                                s0=rs_t[:, b0 + b:b0 + b + 1],
                                s1=0.0,
                                accum_out=sc_t[:, b0 + b:b0 + b + 1],
                            )
                    nc.sync.dma_start(scores_out[s], sc_t[:])

    nc.compile()
    return nc


# ---------------------------------------------------------------------------
# Host-side postprocessing
# ---------------------------------------------------------------------------

def scores_from_results(p, results):
    """results: list of per-core dicts with 'scores' [n_sec,128,rsec//128]."""
    all_scores = np.zeros(p.n_docs, dtype=np.float32)
    for c in range(p.n_cores):
        sc = np.asarray(results[c]["scores"])  # [n_sec, 128, nblk]
        for s in range(p.n_sec):
            r0, r1 = int(p.sec_row_lo[c, s]), int(p.sec_row_hi[c, s])
            nrows = r1 - r0
            flat = sc[s].T.reshape(-1)  # row i at [i%128, i//128] -> transpose
            all_scores[r0:r1] = flat[:nrows]
    return all_scores


def exact_topk(p, approx_scores, crow, indice, values, top_k, n_cand=4096):
    """Pick candidates by approximate score, rescore exactly, return top_k."""
    crow = np.asarray(crow)
    indice = np.asarray(indice)
    values = np.asarray(values)
    n_cand = min(n_cand, p.n_docs)
    cand = np.argpartition(-approx_scores, n_cand - 1)[:n_cand]
    qd = p.query_dense
    exact = np.empty(n_cand, dtype=np.float32)
    for i, d in enumerate(cand):
        s, e = int(crow[d]), int(crow[d + 1])
        exact[i] = np.float32(
            np.sum(values[s:e].astype(np.float32) * qd[indice[s:e]],
                   dtype=np.float32))
    order = np.lexsort((cand, -exact.astype(np.float64)))
    top = order[:top_k]
    return exact[top].astype(np.float32), cand[top].astype(np.int32)


# ---------------------------------------------------------------------------
# SPMD execution via PJRT (axon) with repeat timing
# ---------------------------------------------------------------------------

def run_spmd_timed(nc, in_maps, n_cores=8, n_iters=3):
    """Mirror bass2jax.run_bass_via_pjrt but jit once and time each call.

    Returns (results, times_s): results like run_bass_kernel_spmd
    (list per core of {name: np.ndarray}), times_s = wall time per call.
    """
    import time
    import jax
    from jax.sharding import Mesh, PartitionSpec
    from jax.experimental.shard_map import shard_map
    from concourse import bass2jax, mybir as mb

    bass2jax.install_neuronx_cc_hook()
    assert nc.dbg_addr is None or not nc.dbg_callbacks

    partition_name = nc.partition_id_tensor.name if nc.partition_id_tensor else None
    in_names, out_names, out_avals, zero_outs = [], [], [], []
    for alloc in nc.m.functions[0].allocations:
        if not isinstance(alloc, mb.MemoryLocationSet):
            continue
        name = alloc.memorylocations[0].name
        if alloc.kind == "ExternalInput":
            if name != partition_name:
                in_names.append(name)
        elif alloc.kind == "ExternalOutput":
            shape = tuple(alloc.tensor_shape)
            dtype = mb.dt.np(alloc.dtype)
            out_names.append(name)
            out_avals.append(jax.core.ShapedArray(shape, dtype))
            zero_outs.append(np.zeros(shape, dtype))
    n_params = len(in_names)
    n_outs = len(out_avals)
    in_names_all = in_names + out_names
    if partition_name is not None:
        in_names_all = in_names_all + [partition_name]

    donate = tuple(range(n_params, n_params + n_outs))

    def _body(*args):
        operands = list(args)
        if partition_name is not None:
            operands.append(bass2jax.partition_id_tensor())
        outs = bass2jax._bass_exec_p.bind(
            *operands,
            out_avals=tuple(out_avals),
            in_names=tuple(in_names_all),
            out_names=tuple(out_names),
            lowering_input_output_aliases=(),
            sim_require_finite=True,
            sim_require_nnan=True,
            nc=nc,
        )
        return tuple(outs)

    devices = jax.devices()[:n_cores]
    mesh = Mesh(np.asarray(devices), ("core",))
    in_specs = (PartitionSpec("core"),) * (n_params + n_outs)
    out_specs = (PartitionSpec("core"),) * n_outs
    sharded = jax.jit(
        shard_map(_body, mesh=mesh, in_specs=in_specs, out_specs=out_specs,
                  check_rep=False),
        donate_argnums=donate, keep_unused=True)

    from jax.sharding import NamedSharding
    shd = NamedSharding(mesh, PartitionSpec("core"))
    concat_in = [
        jax.device_put(
            np.concatenate([np.asarray(in_maps[c][name]) for c in range(n_cores)],
                           axis=0), shd)
        for name in in_names
    ]
    jax.block_until_ready(concat_in)
    times = []
    out_arrs = None
    for it in range(n_iters):
        concat_zeros = [
            jax.device_put(
                np.zeros((n_cores * z.shape[0], *z.shape[1:]), z.dtype), shd)
            for z in zero_outs
        ]
        jax.block_until_ready(concat_zeros)
        t0 = time.perf_counter()
        res = sharded(*concat_in, *concat_zeros)
        jax.block_until_ready(res)
        t1 = time.perf_counter()
        times.append(t1 - t0)
        out_arrs = res
    results = [
        {name: np.asarray(out_arrs[i]).reshape(n_cores, *out_avals[i].shape)[c]
         for i, name in enumerate(out_names)}
        for c in range(n_cores)
    ]
    return results, times


def kernel(q_indices, q_values, crow, indice, values, top_k, n_iters=1,
           _cache={}):
    """Full-input kernel: shard internally over 8 cores, return (vals, idx)."""
    VOCAB = 30522
    crow_np = np.asarray(crow)
    ind_np = np.asarray(indice)
    val_np = np.asarray(values)
    p = make_plan(crow_np, np.asarray(q_indices), np.asarray(q_values),
                  VOCAB, n_cores=8, n_sec=5)
    core_ins = make_core_inputs(p, crow_np, ind_np, val_np)
    key = (p.nf, p.rsec, p.n_sec, tuple(p.terms))
    if key in _cache:
        nc = _cache[key]
    else:
        nc = build_program(p, n_devices=8)
        _cache[key] = nc
    try:
        results, times = run_spmd_timed(nc, core_ins, n_cores=8, n_iters=n_iters)
    except Exception:
        # axon workers occasionally desync; one retry after re-jit
        import time as _time
        _time.sleep(5)
        results, times = run_spmd_timed(nc, core_ins, n_cores=8, n_iters=n_iters)
    kernel.last_times = times
    approx = scores_from_results(p, results)
    kernel.last_approx = approx
    vals, idx = exact_topk(p, approx, crow_np, ind_np, val_np, int(top_k))
    return vals, idx



# revision 3
# speedup vs baseline: 85.2437x; 85.2437x over previous
"""nn_CSRSparseRetrievalModel: CSR sparse retrieval (SPLADE-style top-k)
as a Bass/Tile kernel for Trainium2, sharded across 8 NeuronCores.

Strategy (inverted index): the host builds a query-INDEPENDENT CSC
("inverted index") layout of the document matrix, sharded by document
across the 8 cores: for every vocab term v and core c, a fixed-capacity
column holding (doc%128, doc//128, value) of the documents containing v.
Per query, each core:
  1. dma_gather's the 32 query-term columns (transposed so entries land
     one-per-partition),
  2. scales values by the query weights,
  3. scatters entries into a [128, 512] score grid via one-hot matmuls
     accumulated in PSUM: S[doc%128, doc//128] += val*qv  (weights =
     onehot(doc%128), moving = val*qv*onehot(doc//128)),
  4. writes the 62.5K-doc score grid back to DRAM.
The host assembles the 500K approximate scores, picks top candidates and
rescores them exactly in fp32 (same candidate-rescue as the baseline).

kernel(**inputs) -> (top_values float32 [k], top_indices int32 [k])
"""

import numpy as np

import concourse.bass as bass
import concourse.tile as tile
from concourse import bacc, mybir
from concourse.alu_op_type import AluOpType

VOCAB = 30522
N_CORES = 8
CAP = 384           # max entries per (core, vocab) column (asserted)
NSLOT = CAP // 128  # gather slots per field plane
QPAD = 128          # dma_gather transpose needs num_idxs % 128 == 0
NHI = 512           # score grid free dim (>= ceil(62500/128) = 489)


# ---------------------------------------------------------------------------
# Host-side planning
# ---------------------------------------------------------------------------

class Plan:
    pass


def make_plan(crow, q_indices, q_values, vocab, n_cores=8, n_sec=5):
    """Query-side prep: padded query index table (gather idxs wrapped
    layout), replicated query values, iota constants."""
    p = Plan()
    n_docs = crow.shape[0] - 1
    assert n_docs % n_cores == 0
    p.n_cores = n_cores
    p.n_docs = n_docs
    p.dpc = n_docs // n_cores
    assert p.dpc <= 128 * NHI

    qi = np.asarray(q_indices).reshape(-1).astype(np.int64)
    qv = np.asarray(q_values).reshape(-1).astype(np.float64)
    p.n_q = len(qi)
    assert p.n_q <= QPAD

    # dense query for host-side exact rescoring (duplicates coalesced)
    dense = np.zeros(vocab, dtype=np.float64)
    np.add.at(dense, qi, qv)
    p.query_dense = dense.astype(np.float32)
    nz = np.nonzero(dense)[0]
    p.terms = [(int(i), float(np.float32(dense[i]))) for i in nz]

    # gather idx table: idx i lives at [p%16 == i%16, i//16], replicated
    qidx_pad = np.zeros(QPAD, dtype=np.int16)
    qidx_pad[: p.n_q] = qi.astype(np.int16)
    wrapped = qidx_pad.reshape(QPAD // 16, 16).T  # [16, QPAD//16]
    p.qidx_sb = np.tile(wrapped, (8, 1))  # [128, QPAD//16]

    import ml_dtypes
    p.qv_sb = np.broadcast_to(
        qv.astype(ml_dtypes.bfloat16)[None, None, :], (128, NSLOT, p.n_q)
    ).copy()
    p.iota_o = np.broadcast_to(
        np.arange(128, dtype=np.float16)[None, :], (128, 128)).copy()
    p.iota_h = np.broadcast_to(
        np.arange(NHI, dtype=np.float16)[None, :], (128, NHI)).copy()
    p.repeat = 1
    return p


_CSC_CACHE = {}


def build_csc(crow, indice, values, n_cores=N_CORES, cap=CAP, vocab=VOCAB):
    """Query-independent inverted index, sharded by document across cores.

    Returns int16 array [n_cores, vocab, 3*cap]: per column, three planes
    of `cap` int16s: doc%128, doc//128, value (bf16 bit pattern). Unused
    slots are zero (lo=hi=0, val=+0.0 -> contributes nothing).
    """
    import ml_dtypes
    crow = np.asarray(crow, dtype=np.int64)
    ind = np.asarray(indice, dtype=np.int32)
    val = np.asarray(values, dtype=np.float32)
    key_fp = (crow.shape[0], ind.shape[0], int(crow[-1]),
              int(ind[:1000].sum()), float(val[:1000].sum()))
    if key_fp in _CSC_CACHE:
        return _CSC_CACHE[key_fp]

    n_docs = crow.shape[0] - 1
    dpc = n_docs // n_cores
    nnz = ind.shape[0]
    doc = np.repeat(np.arange(n_docs, dtype=np.int32), np.diff(crow))
    order = np.argsort(ind, kind="stable")
    ind_s = ind[order].astype(np.int64)
    doc_s = doc[order]
    val_s = val[order]
    core_s = (doc_s // dpc).astype(np.int64)
    dloc = doc_s - core_s * dpc
    # within a vocab run entries are doc-ascending, hence core-ascending:
    # (vocab, core) groups are contiguous in the sorted order.
    key = ind_s * n_cores + core_s
    counts = np.bincount(key, minlength=vocab * n_cores)
    assert counts.max() <= cap, f"column overflow: {counts.max()} > {cap}"
    starts = np.zeros_like(counts)
    np.cumsum(counts[:-1], out=starts[1:])
    rank = np.arange(nnz, dtype=np.int64) - np.repeat(starts, counts)

    lo16 = (dloc % 128).astype(np.int16)
    hi16 = (dloc // 128).astype(np.int16)
    v16 = val_s.astype(ml_dtypes.bfloat16).view(np.int16)

    csc = np.zeros((n_cores, vocab * 3 * cap), dtype=np.int16)
    flatpos = ind_s * (3 * cap) + rank
    for c in range(n_cores):
        m = core_s == c
        fp = flatpos[m]
        csc[c, fp] = lo16[m]
        csc[c, fp + cap] = hi16[m]
        csc[c, fp + 2 * cap] = v16[m]
    csc = csc.reshape(n_cores, vocab, 3 * cap)
    _CSC_CACHE.clear()
    _CSC_CACHE[key_fp] = csc
    return csc


def make_core_inputs(p, crow, indice, values):
    """Per-core input dicts for the SPMD run."""
    csc = build_csc(crow, indice, values, n_cores=p.n_cores)
    ins = []
    for c in range(p.n_cores):
        ins.append(dict(
            csc=csc[c],
            qidx=p.qidx_sb,
            qv=p.qv_sb,
            iota_o=p.iota_o,
            iota_h=p.iota_h,
        ))
    return ins


# ---------------------------------------------------------------------------
# Device program
# ---------------------------------------------------------------------------

def build_program(p, n_devices=8):
    nc = bacc.Bacc("TRN2", target_bir_lowering=False, debug=False,
                   num_devices=n_devices)
    f32, bf16, fp16 = mybir.dt.float32, mybir.dt.bfloat16, mybir.dt.float16
    i16 = mybir.dt.int16

    csc = nc.declare_dram_parameter("csc", [VOCAB, 3 * CAP], i16, isOutput=False)
    qidx = nc.declare_dram_parameter("qidx", [128, QPAD // 16], i16, isOutput=False)
    qvp = nc.declare_dram_parameter("qv", [128, NSLOT, p.n_q], bf16, isOutput=False)
    iota_o = nc.declare_dram_parameter("iota_o", [128, 128], fp16, isOutput=False)
    iota_h = nc.declare_dram_parameter("iota_h", [128, NHI], fp16, isOutput=False)
    scores_out = nc.declare_dram_parameter("scores", [128, NHI], f32, isOutput=True)

    n_batch = NSLOT * p.n_q

    with tile.TileContext(nc) as tc:
        with tc.tile_pool(name="const", bufs=1) as cpool, \
             tc.tile_pool(name="work", bufs=2) as wpool, \
             tc.tile_pool(name="onehot", bufs=4) as opool, \
             tc.tile_pool(name="ps", bufs=1, space=bass.MemorySpace.PSUM) as ppool:

            qidx_t = cpool.tile([128, QPAD // 16], i16, tag="qidx")
            nc.sync.dma_start(qidx_t[:], qidx[:])
            qv_t = cpool.tile([128, NSLOT, p.n_q], bf16, tag="qv")
            nc.sync.dma_start(qv_t[:], qvp[:])
            io_t = cpool.tile([128, 128], fp16, tag="iota_o")
            nc.sync.dma_start(io_t[:], iota_o[:])
            ih_t = cpool.tile([128, NHI], fp16, tag="iota_h")
            nc.sync.dma_start(ih_t[:], iota_h[:])

            for _rep in range(p.repeat):
                # 1) gather the query columns, transposed: entry e of
                # column t, field plane f -> g[e%128, f*NSLOT + e//128, t]
                g = wpool.tile([128, 3 * NSLOT, QPAD], i16, tag="g")
                nc.gpsimd.dma_gather(
                    g[:], csc[:, :], qidx_t[:],
                    num_idxs=QPAD, num_idxs_reg=QPAD,
                    elem_size=3 * CAP, transpose=True)

                # 2) unpack fields for the used query slots
                lo_f = wpool.tile([128, NSLOT, p.n_q], f32, tag="lo")
                nc.vector.tensor_copy(lo_f[:], g[:, 0:NSLOT, 0:p.n_q])
                hi_f = wpool.tile([128, NSLOT, p.n_q], f32, tag="hi")
                nc.vector.tensor_copy(hi_f[:], g[:, NSLOT:2 * NSLOT, 0:p.n_q])
                sval = wpool.tile([128, NSLOT, p.n_q], f32, tag="sval")
                nc.vector.tensor_tensor(
                    sval[:],
                    g[:, 2 * NSLOT:3 * NSLOT, 0:p.n_q].bitcast(bf16),
                    qv_t[:], AluOpType.mult)

                # 3) one-hot scatter via PSUM-accumulated matmuls
                ps = ppool.tile([128, NHI], f32, tag="ps")
                for b in range(n_batch):
                    t, s = divmod(b, NSLOT)
                    w_t = opool.tile([128, 128], bf16, tag="w")
                    nc.vector.tensor_scalar(
                        w_t[:], io_t[:], lo_f[:, s, t:t + 1], None,
                        AluOpType.is_equal)
                    x_t = opool.tile([128, NHI], bf16, tag="x")
                    nc.vector.tensor_scalar(
                        x_t[:], ih_t[:], hi_f[:, s, t:t + 1],
                        sval[:, s, t:t + 1],
                        AluOpType.is_equal, AluOpType.mult)
                    nc.tensor.matmul(
                        ps[:], w_t[:], x_t[:],
                        start=(b == 0), stop=(b == n_batch - 1))

                # 4) write back the score grid
                out_sb = wpool.tile([128, NHI], f32, tag="out")
                nc.vector.tensor_copy(out_sb[:], ps[:])
                nc.sync.dma_start(scores_out[:], out_sb[:])

    nc.compile()
    return nc


# ---------------------------------------------------------------------------
# Host-side postprocessing
# ---------------------------------------------------------------------------

def scores_from_results(p, results):
    """results: per-core dicts with 'scores' [128, NHI]; local doc d is at
    [d % 128, d // 128]."""
    all_scores = np.zeros(p.n_docs, dtype=np.float32)
    for c in range(p.n_cores):
        sc = np.asarray(results[c]["scores"])  # [128, NHI]
        flat = sc.T.reshape(-1)[: p.dpc]
        all_scores[c * p.dpc:(c + 1) * p.dpc] = flat
    return all_scores


def exact_topk(p, approx_scores, crow, indice, values, top_k, n_cand=4096):
    """Pick candidates by approximate score, rescore exactly, return top_k."""
    crow = np.asarray(crow)
    indice = np.asarray(indice)
    values = np.asarray(values)
    n_cand = min(n_cand, p.n_docs)
    cand = np.argpartition(-approx_scores, n_cand - 1)[:n_cand]
    qd = p.query_dense
    exact = np.empty(n_cand, dtype=np.float32)
    for i, d in enumerate(cand):
        s, e = int(crow[d]), int(crow[d + 1])
        exact[i] = np.float32(
            np.sum(values[s:e].astype(np.float32) * qd[indice[s:e]],
                   dtype=np.float32))
    order = np.lexsort((cand, -exact.astype(np.float64)))
    top = order[:top_k]
    return exact[top].astype(np.float32), cand[top].astype(np.int32)


# ---------------------------------------------------------------------------
# SPMD execution via PJRT (axon) with repeat timing
# ---------------------------------------------------------------------------

def run_spmd_timed(nc, in_maps, n_cores=8, n_iters=3):
    """Mirror bass2jax.run_bass_via_pjrt but jit once and time each call.

    Returns (results, times_s): results like run_bass_kernel_spmd
    (list per core of {name: np.ndarray}), times_s = wall time per call.
    """
    import time
    import jax
    from jax.sharding import Mesh, PartitionSpec
    from jax.experimental.shard_map import shard_map
    from concourse import bass2jax, mybir as mb

    bass2jax.install_neuronx_cc_hook()
    assert nc.dbg_addr is None or not nc.dbg_callbacks

    partition_name = nc.partition_id_tensor.name if nc.partition_id_tensor else None
    in_names, out_names, out_avals, zero_outs = [], [], [], []
    for alloc in nc.m.functions[0].allocations:
        if not isinstance(alloc, mb.MemoryLocationSet):
            continue
        name = alloc.memorylocations[0].name
        if alloc.kind == "ExternalInput":
            if name != partition_name:
                in_names.append(name)
        elif alloc.kind == "ExternalOutput":
            shape = tuple(alloc.tensor_shape)
            dtype = mb.dt.np(alloc.dtype)
            out_names.append(name)
            out_avals.append(jax.core.ShapedArray(shape, dtype))
            zero_outs.append(np.zeros(shape, dtype))
    n_params = len(in_names)
    n_outs = len(out_avals)
    in_names_all = in_names + out_names
    if partition_name is not None:
        in_names_all = in_names_all + [partition_name]

    donate = tuple(range(n_params, n_params + n_outs))

    def _body(*args):
        operands = list(args)
        if partition_name is not None:
            operands.append(bass2jax.partition_id_tensor())
        outs = bass2jax._bass_exec_p.bind(
            *operands,
            out_avals=tuple(out_avals),
            in_names=tuple(in_names_all),
            out_names=tuple(out_names),
            lowering_input_output_aliases=(),
            sim_require_finite=True,
            sim_require_nnan=True,
            nc=nc,
        )
        return tuple(outs)

    devices = jax.devices()[:n_cores]
    mesh = Mesh(np.asarray(devices), ("core",))
    in_specs = (PartitionSpec("core"),) * (n_params + n_outs)
    out_specs = (PartitionSpec("core"),) * n_outs
    sharded = jax.jit(
        shard_map(_body, mesh=mesh, in_specs=in_specs, out_specs=out_specs,
                  check_rep=False),
        donate_argnums=donate, keep_unused=True)

    from jax.sharding import NamedSharding
    shd = NamedSharding(mesh, PartitionSpec("core"))
    concat_in = [
        jax.device_put(
            np.concatenate([np.asarray(in_maps[c][name]) for c in range(n_cores)],
                           axis=0), shd)
        for name in in_names
    ]
    jax.block_until_ready(concat_in)
    times = []
    out_arrs = None
    for it in range(n_iters):
        concat_zeros = [
            jax.device_put(
                np.zeros((n_cores * z.shape[0], *z.shape[1:]), z.dtype), shd)
            for z in zero_outs
        ]
        jax.block_until_ready(concat_zeros)
        t0 = time.perf_counter()
        res = sharded(*concat_in, *concat_zeros)
        jax.block_until_ready(res)
        t1 = time.perf_counter()
        times.append(t1 - t0)
        out_arrs = res
    results = [
        {name: np.asarray(out_arrs[i]).reshape(n_cores, *out_avals[i].shape)[c]
         for i, name in enumerate(out_names)}
        for c in range(n_cores)
    ]
    return results, times


def kernel(q_indices, q_values, crow, indice, values, top_k, n_iters=1,
           _cache={}):
    """Full-input kernel: shard internally over 8 cores, return (vals, idx)."""
    crow_np = np.asarray(crow)
    ind_np = np.asarray(indice)
    val_np = np.asarray(values)
    p = make_plan(crow_np, np.asarray(q_indices), np.asarray(q_values), VOCAB)
    core_ins = make_core_inputs(p, crow_np, ind_np, val_np)
    key = (p.n_q, p.repeat)
    if key in _cache:
        nc = _cache[key]
    else:
        nc = build_program(p, n_devices=8)
        _cache[key] = nc
    try:
        results, times = run_spmd_timed(nc, core_ins, n_cores=8, n_iters=n_iters)
    except Exception:
        # axon workers occasionally desync; one retry after re-jit
        import time as _time
        _time.sleep(5)
        results, times = run_spmd_timed(nc, core_ins, n_cores=8, n_iters=n_iters)
    kernel.last_times = times
    approx = scores_from_results(p, results)
    kernel.last_approx = approx
    vals, idx = exact_topk(p, approx, crow_np, ind_np, val_np, int(top_k))
    return vals, idx
